# revision 1
# baseline (speedup 1.0000x reference)
"""Trainium2 Bass kernel for LocalSpatioTemporalPooling (topk masking).

Reference computation (per sample n):
  x: (N=16, C=256, T=30, H=64, W=32) f32
  ff[n,c,t,s]   = mean over the (8,32) stripe s of the (H,W) plane
  score[n,t,s]  = sum_c ff^2   (monotone in the reference's sqrt/clip score)
  top-2 t per (n,s) by score; output[n, s*256+c] = mean of ff over those 2 t.

Strategy: pure data parallel over batch N across 8 cores (2 samples/core).
Per core:
  phase 1 (memory bound): stream x (125.8 MB) through SBUF in 2 MB tiles,
    vector-reduce each (hs*w)=256-element stripe segment -> stripe sums
    ff_sum (15360, 8), staged in SBUF and spilled once to a DRAM scratch.
  phase 2 (tiny): reload ff as (c-partition, n*t*s) tiles, square + ones-matmul
    on PE -> per-(n,t,s) score; top-2 via DVE max8; mask = (score >= 2nd max)
    scaled by 1/512 (fuses the /2 top-k mean and the /256 stripe mean);
    broadcast mask over partitions via PE outer product; masked reduce over t
    -> output (2, 2048) in (ci, c_local, s) column order, reordered on host.
"""

import sys
from contextlib import ExitStack

for _p in ("/opt/trn_rl_repo",):
    if _p not in sys.path:
        sys.path.insert(0, _p)

import numpy as np

import concourse.bass as bass
import concourse.tile as tile
from concourse import bacc, mybir
from concourse.bass_utils import run_bass_kernel_spmd

N_CORES = 8
N, C, T, H, W = 16, 256, 30, 64, 32
S = 8          # stripes
HSW = (H // S) * W   # 256 elements per stripe
NL = N // N_CORES    # samples per core = 2
R = NL * C * T       # rows per core = 15360, one row = one (n,c,t) plane
HW = H * W           # 2048
ROWS_PER_TILE = 256  # -> (128 partitions, 2 rows each) = 2 MB f32 tile
NTILES = R // ROWS_PER_TILE  # 60
FREE = (ROWS_PER_TILE // 128) * HW  # 4096
TS = T * S           # 240
F2 = NL * TS         # 480
OUT_COLS = S * C     # 2048
F32 = mybir.dt.float32
X = mybir.AxisListType.X


def build_program() -> bacc.Bacc:
    nc = bacc.Bacc("TRN2", target_bir_lowering=False, debug=False,
                   num_devices=N_CORES)
    x = nc.dram_tensor("x", [R, HW], F32, kind="ExternalInput").ap()
    out = nc.dram_tensor("out", [NL, OUT_COLS], F32, kind="ExternalOutput").ap()

    with tile.TileContext(nc) as tc, ExitStack() as ctx:
        xpool = ctx.enter_context(tc.tile_pool(name="xtiles", bufs=4))
        cpool = ctx.enter_context(tc.tile_pool(name="consts", bufs=1))
        spool = ctx.enter_context(tc.tile_pool(name="small", bufs=1))
        ppool = ctx.enter_context(tc.tile_pool(name="psum", bufs=1, space="PSUM"))
        dram = ctx.enter_context(tc.tile_pool(name="dram", bufs=1, space="DRAM"))

        # ---- phase 1: per-stripe sums -> ff_all (SBUF) -> ff_dram ----
        ff_all = cpool.tile([128, NTILES * 16], F32)  # col = i*16 + k*8 + s
        for i in range(NTILES):
            xt = xpool.tile([128, FREE], F32, name="xt", tag="xt")
            eng = nc.sync if (i % 2 == 0) else nc.scalar
            eng.dma_start(
                xt[:],
                x[i * ROWS_PER_TILE:(i + 1) * ROWS_PER_TILE, :]
                .rearrange("(p k) m -> p (k m)", p=128),
            )
            nc.vector.reduce_sum(
                ff_all[:, i * 16:(i + 1) * 16],
                xt[:].rearrange("p (k s e) -> p k s e", k=2, s=S),
                axis=X,
            )

        # linear layout of ff_dram[row, s] matches ff_all[(p), (i,k,s)]
        ff_dram = dram.tile([R, S], F32)
        nc.sync.dma_start(
            ff_dram.rearrange("(i p k) s -> p i k s", i=NTILES, p=128, k=2),
            ff_all[:],
        )

        # ---- phase 2: scores, top-2 mask, masked mean ----
        ones_col = cpool.tile([128, 1], F32)   # K=128 stationary: column of ones
        nc.vector.memset(ones_col[:], 1.0)
        ones_row = cpool.tile([1, 128], F32)   # K=1 stationary: row of 1/512
        nc.vector.memset(ones_row[:], 1.0 / 512.0)

        # ff by channel: partition = c (two halves), free = (n, t*s)
        ffq = []
        ff_by_c = ff_dram.rearrange("(n c t) s -> c n (t s)", n=NL, c=C)
        for ci in range(2):
            fq = spool.tile([128, NL, TS], F32, name=f"ffq{ci}", tag=f"ffq{ci}")
            nc.sync.dma_start(fq[:], ff_by_c[ci * 128:(ci + 1) * 128])
            ffq.append(fq)

        psc = ppool.tile([1, F2], F32, name="psc", tag="psc")  # sum_c ff^2
        for ci in range(2):
            sq = spool.tile([128, F2], F32, name=f"sq{ci}", tag=f"sq{ci}")
            nc.scalar.square(sq[:], ffq[ci][:].rearrange("p a b -> p (a b)"))
            nc.tensor.matmul(psc[:], ones_col[:], sq[:],
                             start=(ci == 0), stop=(ci == 1))

        sc_sb = spool.tile([1, F2], F32, name="sc_sb")
        nc.scalar.copy(sc_sb[:], psc[:])

        # top-2 mask per (n, s) segment, computed in place on one partition.
        # seg: (q, n, s, t) view for per-(n,s) reduces over t;
        # v4/bc: matched 4D (q, n, t, s) iteration, bc has stride-0 over t.
        def seg(ap):
            return ap.rearrange("q (n t s) -> q n s t", n=NL, t=T, s=S)

        def v4(ap):
            return ap.rearrange("q (n t s) -> q n t s", n=NL, t=T, s=S)

        def bc(ap):
            return (ap.rearrange("q (n s) -> q n s", n=NL)[:, :, None, :]
                    .broadcast_to((1, NL, T, S)))

        ge = mybir.AluOpType.is_ge
        m1 = spool.tile([1, NL * S], F32, name="m1")
        nc.vector.reduce_max(m1[:], seg(sc_sb[:]), axis=X)
        eqb = spool.tile([1, F2], F32, name="eqb")
        nc.vector.tensor_tensor(v4(eqb[:]), v4(sc_sb[:]), bc(m1[:]), op=ge)
        nc.vector.tensor_scalar(eqb[:], eqb[:], 1e30, None,
                                op0=mybir.AluOpType.mult)
        tmp = spool.tile([1, F2], F32, name="tmp")
        nc.vector.tensor_tensor(tmp[:], sc_sb[:], eqb[:],
                                op=mybir.AluOpType.subtract)
        m2 = spool.tile([1, NL * S], F32, name="m2")
        nc.vector.reduce_max(m2[:], seg(tmp[:]), axis=X)
        maskrow = spool.tile([1, F2], F32, name="maskrow")
        nc.vector.tensor_tensor(v4(maskrow[:]), v4(sc_sb[:]), bc(m2[:]), op=ge)

        # broadcast mask to all 128 partitions scaled by 1/512 (the 1/2 top-k
        # mean * 1/256 stripe mean): (1/512)ones(1,128).T @ maskrow(1,480)
        psb = ppool.tile([128, F2], F32, name="psb", tag="psb")
        nc.tensor.matmul(psb[:], ones_row[:], maskrow[:], start=True, stop=True)

        for ci in range(2):
            prod = spool.tile([128, F2], F32, name=f"prod{ci}", tag=f"prod{ci}")
            nc.vector.tensor_tensor(prod[:], ffq[ci][:].rearrange("p a b -> p (a b)"),
                                    psb[:], op=mybir.AluOpType.mult)
            red = spool.tile([128, NL * S], F32, name=f"red{ci}", tag=f"red{ci}")
            nc.vector.reduce_sum(
                red[:], prod[:].rearrange("p (n t s) -> p n s t", n=NL, t=T, s=S),
                axis=X,
            )
            for n_ in range(NL):
                nc.sync.dma_start(
                    out[n_, ci * 1024:(ci + 1) * 1024]
                    .rearrange("(p s) -> p s", p=128),
                    red[:, n_ * S:(n_ + 1) * S],
                )

    nc.compile()
    return nc


_NC_CACHE: list = []


def _get_program() -> bacc.Bacc:
    if not _NC_CACHE:
        _NC_CACHE.append(build_program())
    return _NC_CACHE[0]


def kernel(x: np.ndarray) -> np.ndarray:
    assert x.shape == (N, C, T, H, W), x.shape
    nc = _get_program()
    xf = np.ascontiguousarray(x, dtype=np.float32)
    in_maps = [
        {"x": xf[i * NL:(i + 1) * NL].reshape(R, HW)} for i in range(N_CORES)
    ]
    res = run_bass_kernel_spmd(nc, in_maps, core_ids=list(range(N_CORES)))
    parts = [res.results[i]["out"] for i in range(N_CORES)]
    raw = np.concatenate(parts, axis=0)  # (16, 2048), col = ci*1024 + cl*8 + s
    # reorder columns to the reference's s*256 + (ci*128 + cl)
    full = raw.reshape(N, 2, 128, S).transpose(0, 3, 1, 2).reshape(N, OUT_COLS)
    return np.ascontiguousarray(full)



# revision 3
# speedup vs baseline: 1.1556x; 1.1556x over previous
"""Trainium2 Bass kernel for LocalSpatioTemporalPooling (topk masking), v2.

Reference computation (per sample n):
  x: (N=16, C=256, T=30, H=64, W=32) f32
  ff[n,c,t,s]   = mean over the (8,32) stripe s of the (H,W) plane
  score[n,t,s]  = sum_c ff^2   (monotone in the reference's sqrt/clip score)
  top-2 t per (n,s) by score; output[n, s*256+c] = mean of ff over those 2 t.

v2 changes vs baseline:
  - Larger stream tiles (6 MB, 768 rows) for the first 3 quarters; the last
    quarter uses 2 MB tiles so the final reduce tail is short. All stream
    tiles share one pool tag so buffer rotation sequences their DMAs behind
    the reduces that free the slots.
  - Each tile's load is split across both HWDGE queues (half the rows each)
    so the queues drain one tile together and buffers free in tile order.
  - ff scratch spill/reload runs on the gpsimd (SWDGE) queue per quarter
    (n, c-half), overlapping the stream; the last quarter uses the HWDGE
    queues (idle by then, lower latency).
  - Phase 2 (scores, top-2 mask, masked mean, output) is split per sample n:
    sample 0's entire tail runs mid-stream after quarter 1.
"""

import sys
from contextlib import ExitStack

for _p in ("/opt/trn_rl_repo",):
    if _p not in sys.path:
        sys.path.insert(0, _p)

import numpy as np

import concourse.bass as bass
import concourse.tile as tile
from concourse import bacc, mybir
from concourse.bass_utils import run_bass_kernel_spmd

N_CORES = 8
N, C, T, H, W = 16, 256, 30, 64, 32
S = 8            # stripes
NL = N // N_CORES            # samples per core = 2
R = NL * C * T               # rows per core = 15360; row = (n, c, t)
HW = H * W                   # 2048
QROWS = R // 4               # 3840 rows per quarter; quarter j = (n, c-half)
BIG_ROWS = 768               # 6 MB tiles (k=6) for quarters 0-2
BIG_PER_Q = QROWS // BIG_ROWS        # 5
SMALL_ROWS = 256             # 2 MB tiles (k=2) for quarter 3 (short tail)
SMALL_PER_Q = QROWS // SMALL_ROWS    # 15
TS = T * S                   # 240
F2 = NL * TS                 # 480
OUT_COLS = S * C             # 2048
F32 = mybir.dt.float32
X = mybir.AxisListType.X


def build_program() -> bacc.Bacc:
    nc = bacc.Bacc("TRN2", target_bir_lowering=False, debug=False,
                   num_devices=N_CORES)
    x = nc.dram_tensor("x", [R, HW], F32, kind="ExternalInput").ap()
    out = nc.dram_tensor("out", [NL, OUT_COLS], F32, kind="ExternalOutput").ap()

    with tile.TileContext(nc) as tc, ExitStack() as ctx:
        xpool = ctx.enter_context(tc.tile_pool(name="xtiles", bufs=8))
        cpool = ctx.enter_context(tc.tile_pool(name="consts", bufs=1))
        spool = ctx.enter_context(tc.tile_pool(name="aux", bufs=1))
        ppool = ctx.enter_context(tc.tile_pool(name="psum", bufs=1, space="PSUM"))
        dram = ctx.enter_context(tc.tile_pool(name="dram", bufs=1, space="DRAM"))

        ones_col = cpool.tile([128, 1], F32)   # K=128 stationary: column of ones
        nc.vector.memset(ones_col[:], 1.0)
        ones_row = cpool.tile([1, 128], F32)   # K=1 stationary: row of 1/512
        nc.vector.memset(ones_row[:], 1.0 / 512.0)

        ff_q = [dram.tile([QROWS, S], F32, name=f"ff_q{j}") for j in range(4)]
        ffq_sb = []   # per-quarter ff, partition = c_local, free = (t, s)
        psc = ppool.tile([1, F2], F32, name="psc", tag="psc")  # sum_c ff^2
        psb = ppool.tile([128, F2], F32, name="psb", tag="psb")
        red = [spool.tile([128, NL * S], F32, name=f"red{ci}") for ci in range(2)]

        ge = mybir.AluOpType.is_ge

        def phase2_tail(n_):
            """Top-2 mask + masked mean + output for sample n_. Requires
            psc[:, n_*TS:(n_+1)*TS] complete and ffq_sb[2*n_ .. 2*n_+1]."""
            lo, hi = n_ * TS, (n_ + 1) * TS
            sc = spool.tile([1, TS], F32, name=f"sc{n_}")
            nc.scalar.copy(sc[:], psc[:, lo:hi])

            def seg(ap):  # (q, s, t): per-s reduce over t
                return ap.rearrange("q (t s) -> q s t", t=T, s=S)

            def v3(ap):   # matched 3D iteration (q, t, s)
                return ap.rearrange("q (t s) -> q t s", t=T, s=S)

            def bc(ap):   # broadcast (q, s) over t
                return ap[:, None, :].broadcast_to((1, T, S))

            # scores are sums of squares (> 0), so zeroing the max position
            # (multiply by the sc<m1 indicator) makes it the minimum; the
            # next reduce_max then finds the second-largest score.
            m1 = spool.tile([1, S], F32, name=f"m1_{n_}")
            nc.vector.reduce_max(m1[:], seg(sc[:]), axis=X)
            ltb = spool.tile([1, TS], F32, name=f"ltb{n_}")
            nc.vector.tensor_tensor(v3(ltb[:]), v3(sc[:]), bc(m1[:]),
                                    op=mybir.AluOpType.is_lt)
            tmp = spool.tile([1, TS], F32, name=f"tmp{n_}")
            nc.vector.tensor_tensor(tmp[:], sc[:], ltb[:],
                                    op=mybir.AluOpType.mult)
            m2 = spool.tile([1, S], F32, name=f"m2_{n_}")
            nc.vector.reduce_max(m2[:], seg(tmp[:]), axis=X)
            maskrow = spool.tile([1, TS], F32, name=f"maskrow{n_}")
            nc.vector.tensor_tensor(v3(maskrow[:]), v3(sc[:]), bc(m2[:]), op=ge)

            # broadcast mask to 128 partitions scaled by 1/512 (fuses the
            # /2 top-k mean and the /256 stripe mean)
            nc.tensor.matmul(psb[:, lo:hi], ones_row[:], maskrow[:],
                             start=True, stop=True)

            for ci in range(2):
                j = n_ * 2 + ci
                prod = spool.tile([128, TS], F32, name=f"prod{j}")
                nc.vector.tensor_tensor(prod[:], ffq_sb[j][:], psb[:, lo:hi],
                                        op=mybir.AluOpType.mult)
                nc.vector.reduce_sum(
                    red[ci][:, n_ * S:(n_ + 1) * S],
                    prod[:].rearrange("c (t s) -> c s t", t=T, s=S),
                    axis=X,
                )
                (nc.sync if ci == 0 else nc.scalar).dma_start(
                    out[n_, ci * 1024:(ci + 1) * 1024]
                    .rearrange("(p s) -> p s", p=128),
                    red[ci][:, n_ * S:(n_ + 1) * S],
                )

        for j in range(4):
            n_, ci = j // 2, j % 2
            # (count, rows) segments per quarter; the last quarter tapers to
            # short tiles so the final reduce + spill tail is minimal.
            segs = [(30, 128)]
            ffacc = spool.tile([128, TS], F32, name=f"ffacc{j}")
            spill_eng = nc.gpsimd if j < 3 else nc.sync
            reload_eng = nc.gpsimd if j < 3 else nc.scalar
            row_off = 0   # local row within the quarter
            col_off = 0   # column within ffacc
            gi = 0        # tile index within the quarter
            # 30 tiles of 128 rows (1 MB): partition p maps directly to
            # row r0+p, so no grouped/rearranged APs with degenerate size-1
            # dims are needed anywhere on this path.
            for count, rows in segs:
                assert rows == 128
                seg_col0 = col_off
                for i in range(count):
                    r0 = j * QROWS + row_off + i * rows
                    xt = xpool.tile([128, HW], F32, name=f"xt{j}", tag="xt")
                    eng = nc.sync if (gi % 2 == 0) else nc.scalar
                    eng.dma_start(xt[:], x[r0:r0 + 128, :])
                    nc.vector.reduce_sum(
                        ffacc[:, col_off:col_off + S],
                        xt[:].rearrange("p (s e) -> p s e", s=S),
                        axis=X,
                    )
                    col_off += S
                    gi += 1
                    if j == 2 and gi == 3:
                        # sample 0's whole tail, emitted a few tiles into
                        # quarter 2: its inputs (psc cols 0:TS, ffq 0/1)
                        # completed during quarter 1, so the in-order DVE
                        # pops these ops without waiting and absorbs them
                        # in its per-tile slack.
                        phase2_tail(0)
                # spill this segment's ff to DRAM scratch
                # (local row = row_off + i*128 + p)
                spill_eng.dma_start(
                    ff_q[j][row_off:row_off + count * 128, :]
                    .rearrange("(i p) s -> p i s", i=count, p=128),
                    ffacc[:, seg_col0:col_off],
                )
                row_off += count * 128
            # reload with partition = c_local (row within quarter = c_local*T + t)
            fq = spool.tile([128, TS], F32, name=f"ffq{j}")
            reload_eng.dma_start(
                fq[:],
                ff_q[j].rearrange("(c t) s -> c (t s)", c=128, t=T),
            )
            ffq_sb.append(fq)
            s_ = spool.tile([128, TS], F32, name=f"sq{j}")
            nc.scalar.square(s_[:], fq[:])
            nc.tensor.matmul(psc[:, n_ * TS:(n_ + 1) * TS], ones_col[:], s_[:],
                             start=(ci == 0), stop=(ci == 1))
        phase2_tail(1)

    nc.compile()
    return nc


_NC_CACHE: list = []


def _get_program() -> bacc.Bacc:
    if not _NC_CACHE:
        _NC_CACHE.append(build_program())
    return _NC_CACHE[0]


def kernel(x: np.ndarray) -> np.ndarray:
    assert x.shape == (N, C, T, H, W), x.shape
    nc = _get_program()
    xf = np.ascontiguousarray(x, dtype=np.float32)
    in_maps = [
        {"x": xf[i * NL:(i + 1) * NL].reshape(R, HW)} for i in range(N_CORES)
    ]
    res = run_bass_kernel_spmd(nc, in_maps, core_ids=list(range(N_CORES)))
    parts = [res.results[i]["out"] for i in range(N_CORES)]
    raw = np.concatenate(parts, axis=0)  # (16, 2048), col = ci*1024 + cl*8 + s
    # reorder columns to the reference's s*256 + (ci*128 + cl)
    full = raw.reshape(N, 2, 128, S).transpose(0, 3, 1, 2).reshape(N, OUT_COLS)
    return np.ascontiguousarray(full)
